# revision 1
# baseline (speedup 1.0000x reference)
"""Trainium2 Bass kernel for nn_PredictionNet — data-parallel over batch.

Each of the 8 cores handles a 32-sample batch slice with ALL expert weights
(fp16, ~17.7MB, fully SBUF-resident). No cross-core communication.

Per layer:  y_e = h @ W_e + b_e  for each expert (PE, fp16 in / fp32 PSUM),
blended with per-sample coefs via fused DVE scalar_tensor_tensor chains
(coef is a per-partition scalar in the [batch, out] orientation), ELU on
ACT+DVE, then a PE transpose puts the hidden state back K-major for the
next layer's stationary operand. Bias rides a K=1 matmul against the ones
row of the [ones; z^T] tile (also the z-chunk of layers 2/3).
"""

import sys

sys.path.insert(0, "/opt/trn_rl_repo")

import numpy as np

import concourse.bass as bass
import concourse.mybir as mybir
import concourse.tile as tile
from concourse.bass_utils import run_bass_kernel_spmd

B, E = 256, 6
IN, HID, OUT, ZD = 1664, 512, 618, 32
N_CORES = 8
CORE_IDS = list(range(N_CORES))
BC = B // N_CORES         # 32 batch rows per core
K1 = IN // 128            # 13 k-chunks, layer 1
KH = HID // 128           # 4 k-chunks for the hidden part of layers 2/3
OUTP = 640                # layer-3 output padded 618 -> 640
NH3 = 2                   # layer-3 output split into halves of 320 (psum bank)
OH3 = OUTP // NH3
FP32 = mybir.dt.float32
FP16 = mybir.dt.float16


def _split_waits(nc, max_waits=1):
    """neuronxcc walrus accepts only ONE sync-wait per instruction: hoist
    extras onto same-engine NoOps placed before the offending instruction."""
    n = 0
    for fn in nc.m.functions:
        for blk in fn.blocks:
            insts = blk.instructions
            if not any(
                i.sync_info is not None and len(i.sync_info.on_wait) > max_waits
                for i in insts
            ):
                continue
            out = []
            for inst in insts:
                si = inst.sync_info
                if si is not None and len(si.on_wait) > max_waits:
                    for w in si.on_wait[:-max_waits]:
                        n += 1
                        nop = mybir.InstNoOp(name=f"I-wfix{n}", ins=[], outs=[])
                        nop.engine = inst.engine
                        nop.sync_info = mybir.SyncInfo(on_wait=[w], on_update=[])
                        try:
                            nc.register_instruction(nop, overwrite=True)
                        except Exception:
                            pass
                        out.append(nop)
                    inst.sync_info = mybir.SyncInfo(
                        on_wait=list(si.on_wait[-max_waits:]),
                        on_update=list(si.on_update),
                    )
                out.append(inst)
            blk.instructions = out
    return n


def build_nc():
    nc = bass.Bass()

    hc_d = nc.dram_tensor("hc", [128, K1, BC], FP16, kind="ExternalInput")
    zc_d = nc.dram_tensor("zc", [1 + ZD, BC], FP16, kind="ExternalInput")
    coefc_d = nc.dram_tensor("coefc", [BC, E], FP32, kind="ExternalInput")
    idn_d = nc.dram_tensor("idn", [BC, BC], FP32, kind="ExternalInput")
    w1_d = nc.dram_tensor("w1", [E, 128, K1, HID], FP16, kind="ExternalInput")
    b1_d = nc.dram_tensor("b1", [1, E, HID], FP16, kind="ExternalInput")
    w2z_d = nc.dram_tensor("w2z", [E, 1 + ZD, HID], FP16, kind="ExternalInput")
    w2_d = nc.dram_tensor("w2", [E, 128, KH, HID], FP16, kind="ExternalInput")
    w3z_d = nc.dram_tensor("w3z", [E, 1 + ZD, OUTP], FP16, kind="ExternalInput")
    w3_d = nc.dram_tensor("w3", [E, 128, KH, OUTP], FP16, kind="ExternalInput")
    out_d = nc.dram_tensor("outc", [BC, OUTP], FP32, kind="ExternalOutput")

    with tile.TileContext(nc) as tc:
        with (
            tc.tile_pool(name="const", bufs=1) as cp,
            tc.tile_pool(name="work", bufs=1) as wp,
            tc.tile_pool(name="psum", bufs=3, space="PSUM") as pp,
            tc.tile_pool(name="psumt", bufs=2, space="PSUM") as pt,
        ):
            # ---- big weights first: one DMA per expert per layer, all on
            # the sync HWDGE queue in layer order (the SDMA engines round-
            # robin concurrent transfers, so layer order = arrival order) ----
            w1 = cp.tile([128, E, K1, HID], FP16)
            for e in range(E):
                eng = nc.sync if e % 2 == 0 else nc.scalar
                eng.dma_start(out=w1[:, e, :, :], in_=w1_d[e])
            # ---- small resident inputs on the other queues ----
            hc = cp.tile([128, K1, BC], FP16)
            nc.gpsimd.dma_start(out=hc[:], in_=hc_d[:])
            zc = cp.tile([1 + ZD, BC], FP16)
            nc.gpsimd.dma_start(out=zc[:], in_=zc_d[:])
            coefc = cp.tile([BC, E], FP32)
            nc.gpsimd.dma_start(out=coefc[:], in_=coefc_d[:])
            idn = cp.tile([BC, BC], FP32)
            nc.gpsimd.dma_start(out=idn[:], in_=idn_d[:])
            b1 = cp.tile([1, E, HID], FP16)
            nc.gpsimd.dma_start(out=b1[:], in_=b1_d[:])
            w2z = cp.tile([1 + ZD, E, HID], FP16)
            nc.gpsimd.dma_start(
                out=w2z[:], in_=w2z_d.rearrange("e z o -> z e o")
            )
            w3z = cp.tile([1 + ZD, E, OUTP], FP16)
            nc.gpsimd.dma_start(
                out=w3z[:], in_=w3z_d.rearrange("e z o -> z e o")
            )
            w2 = cp.tile([128, E, KH, HID], FP16)
            for e in range(E):
                eng = nc.sync if e % 2 == 0 else nc.scalar
                eng.dma_start(out=w2[:, e, :, :], in_=w2_d[e])
            w3 = cp.tile([128, E, KH, OUTP], FP16)
            for e in range(E):
                eng = nc.sync if e % 2 == 0 else nc.scalar
                eng.dma_start(out=w3[:, e, :, :], in_=w3_d[e])

            def blend_step(acc_prev, ps, e, tag):
                """acc = ps * coef[:, e] + acc_prev (fused on DVE); returns acc."""
                acc = wp.tile([BC, ps.shape[-1]], FP32, name=f"{tag}_acc{e}", tag=f"{tag}_acc", bufs=2)
                if acc_prev is None:
                    nc.vector.tensor_scalar(
                        acc[:], ps[:], coefc[:, e : e + 1], None,
                        mybir.AluOpType.mult,
                    )
                else:
                    nc.vector.scalar_tensor_tensor(
                        acc[:], ps[:], coefc[:, e : e + 1], acc_prev[:],
                        mybir.AluOpType.mult, mybir.AluOpType.add,
                    )
                return acc

            def elu(acc, tag):
                tneg = wp.tile([BC, HID], FP32, tag=f"{tag}_neg")
                nc.vector.tensor_scalar_min(tneg[:], acc[:], 0.0)
                texp = wp.tile([BC, HID], FP32, tag=f"{tag}_exp")
                nc.scalar.activation(
                    texp[:], tneg[:], mybir.ActivationFunctionType.Exp
                )
                trel = wp.tile([BC, HID], FP32, tag=f"{tag}_rel")
                nc.scalar.activation(
                    trel[:], acc[:], mybir.ActivationFunctionType.Relu
                )
                res = wp.tile([BC, HID], FP32, tag=f"{tag}_res")
                nc.vector.scalar_tensor_tensor(
                    res[:], texp[:], -1.0, trel[:],
                    mybir.AluOpType.add, mybir.AluOpType.add,
                )
                return res

            def transpose_kmajor(h, tag):
                """h [32, 512] fp32 -> fp16 K-major [128, 4, 32]."""
                ht = wp.tile([128, KH, BC], FP16, name=f"{tag}_ht", tag=f"{tag}_ht")
                for j in range(KH):
                    ps = pt.tile([128, BC], FP32, name=f"{tag}_tp{j}", tag="tpose")
                    nc.tensor.transpose(
                        ps[:], h[:, j * 128 : (j + 1) * 128], idn[:]
                    )
                    nc.vector.tensor_copy(ht[:, j, :], ps[:])
                return ht

            # ================= Layer 1 =================
            acc = None
            for e in range(E):
                ps = pp.tile([BC, HID], FP32, name=f"l1ps{e}", tag="ps")
                nc.tensor.matmul(
                    ps[:], zc[0:1, :], b1[0:1, e, :], start=True, stop=False
                )
                for k in range(K1):
                    nc.tensor.matmul(
                        ps[:], hc[:, k, :], w1[:, e, k, :],
                        start=False, stop=(k == K1 - 1),
                    )
                acc = blend_step(acc, ps, e, "l1")
            h1 = elu(acc, "l1")
            h1t = transpose_kmajor(h1, "l1")

            # ================= Layer 2 =================
            acc = None
            for e in range(E):
                ps = pp.tile([BC, HID], FP32, name=f"l2ps{e}", tag="ps")
                nc.tensor.matmul(
                    ps[:], zc[:], w2z[:, e, :], start=True, stop=False
                )
                for k in range(KH):
                    nc.tensor.matmul(
                        ps[:], h1t[:, k, :], w2[:, e, k, :],
                        start=False, stop=(k == KH - 1),
                    )
                acc = blend_step(acc, ps, e, "l2")
            h2 = elu(acc, "l2")
            h2t = transpose_kmajor(h2, "l2")

            # ================= Layer 3 =================
            res3 = wp.tile([BC, OUTP], FP32, tag="res3")
            for half in range(NH3):
                sl = slice(half * OH3, (half + 1) * OH3)
                acc = None
                for e in range(E):
                    ps = pp.tile([BC, OH3], FP32, name=f"l3ps{half}_{e}", tag="ps")
                    nc.tensor.matmul(
                        ps[:], zc[:], w3z[:, e, sl], start=True, stop=False
                    )
                    for k in range(KH):
                        nc.tensor.matmul(
                            ps[:], h2t[:, k, :], w3[:, e, k, sl],
                            start=False, stop=(k == KH - 1),
                        )
                    acc = blend_step(acc, ps, e, f"l3h{half}")
                nc.vector.tensor_copy(res3[:, sl], acc[:])
            nc.sync.dma_start(out=out_d[:], in_=res3[:])

    _split_waits(nc)
    _trim_tail(nc)
    return nc


def _trim_tail(nc):
    """Drop the second all-engine barrier round + sem-clear at the kernel
    tail: the first drain+barrier already guarantees completion, and the
    preamble re-initializes semaphores on any re-execution (verified by
    double-execution test)."""
    blk = nc.m.functions[0].blocks[-1]
    insts = blk.instructions
    cut = None
    for idx in range(len(insts) - 1, -1, -1):
        if type(insts[idx]).__name__ == "InstISA":
            cut = idx
            break
    if cut is not None:
        blk.instructions = insts[:cut]


_NC_CACHE = None


def _get_nc():
    global _NC_CACHE
    if _NC_CACHE is None:
        _NC_CACHE = build_nc()
    return _NC_CACHE


def make_in_maps(p_prev, blending_coef, z, w_l1, b_l1, w_l2, b_l2, w_l3, b_l3):
    f, h = np.float32, np.float16
    h0 = np.concatenate([z, p_prev], axis=1).astype(f)          # [B, IN]

    w1 = np.ascontiguousarray(                                   # [E,128,K1,HID]
        w_l1.astype(h).reshape(E, K1, 128, HID).transpose(0, 2, 1, 3)
    )
    b1 = b_l1.astype(h)[None]                                    # [1, E, HID]
    w2z = np.concatenate(                                        # [E, 33, HID]
        [b_l2.astype(h)[:, None, :], w_l2[:, :ZD, :].astype(h)], axis=1
    )
    w2 = np.ascontiguousarray(                                   # [E,128,KH,HID]
        w_l2[:, ZD:, :].astype(h).reshape(E, KH, 128, HID).transpose(0, 2, 1, 3)
    )
    w3pad = np.zeros((E, HID + ZD, OUTP), h)
    w3pad[:, :, :OUT] = w_l3.astype(h)
    b3pad = np.zeros((E, OUTP), h)
    b3pad[:, :OUT] = b_l3.astype(h)
    w3z = np.concatenate([b3pad[:, None, :], w3pad[:, :ZD, :]], axis=1)
    w3 = np.ascontiguousarray(
        w3pad[:, ZD:, :].reshape(E, KH, 128, OUTP).transpose(0, 2, 1, 3)
    )
    idn = np.eye(BC, dtype=f)

    in_maps = []
    for c in range(N_CORES):
        bs = slice(c * BC, (c + 1) * BC)
        hc = np.ascontiguousarray(
            h0[bs].T.reshape(K1, 128, BC).transpose(1, 0, 2)
        ).astype(h)                                              # [128, K1, BC]
        zcc = np.concatenate(
            [np.ones((1, BC), f), z[bs].T.astype(f)], axis=0
        ).astype(h)                                              # [33, BC]
        coefc = np.ascontiguousarray(blending_coef[bs].astype(f))  # [BC, E]
        in_maps.append(
            {
                "hc": hc, "zc": zcc, "coefc": coefc, "idn": idn,
                "w1": w1, "b1": b1, "w2z": w2z, "w2": w2,
                "w3z": w3z, "w3": w3,
            }
        )
    return in_maps


def assemble_output(results):
    full = np.concatenate(
        [results[c]["outc"] for c in range(N_CORES)], axis=0
    )                                                            # [256, 640]
    return np.ascontiguousarray(full[:, :OUT]).astype(np.float32)


def kernel(p_prev, blending_coef, z, w_l1, b_l1, w_l2, b_l2, w_l3, b_l3):
    args = [
        np.asarray(a)
        for a in (p_prev, blending_coef, z, w_l1, b_l1, w_l2, b_l2, w_l3, b_l3)
    ]
    nc = _get_nc()
    in_maps = make_in_maps(*args)
    res = run_bass_kernel_spmd(nc, in_maps, CORE_IDS)
    return assemble_output(res.results)

